# revision 1
# baseline (speedup 1.0000x reference)
"""Trainium2 Bass kernel for the MERU-style hyperbolic contrastive loss.

Problem (hardcoded shapes):
  text_embeddings (8192, 768) f32, label_embeddings (4096, 768) f32,
  target_labels (8192,) int32, three scalar log-params.
  Output: loss (8192,) f32 per-sample.

Sharding: data-parallel over text rows across 8 NeuronCores (1024 rows each);
label_embeddings and scalars replicated.

Per-core algorithm (v2 — grouped pipeline, positive masked in the matmul):
  Lorentz inner product factors as  inner[m,c] = hx_m * W[m,c] with
      W[m,c] = g_c*S_raw[m,c] - xtq_m*yt_c,
  where S_raw = raw_text @ raw_labels^T, hx_m/g_c the exp-map scale factors,
  yt_c the label time component and xtq_m = x_time_m / hx_m.  Since hx_m > 0
  is row-constant, per-row top-k runs directly on W in PSUM (no rescale).

   1. Text: one staged f32 load; per-row norms; Sqrt-free stats (Exp/Ln only,
      single ACT table); bf16 round-trip through DRAM with XBAR DMA-transpose
      for the K-major layout; xtq hi/lo rank-3 rows folded into the matmul.
   2. Labels in 4 pipelined groups of 1024: grouped DMA load, norms, group
      stats, then the f32->bf16 conversion is fused with the g_c scale so the
      matmul computes g_c*S_raw directly; group transposes stream to yT.
   3. Main loop per (group, m-tile): 2x(6 bf16 K=128 + rank-3 + mask) matmuls
      into a 2-bank PSUM tile.  The mask matmul adds -448*onehot(target) so
      the positive can never appear among the top-8; onehot rows are built on
      DVE from a u16 iota at 4x rate.  DVE max8 reads W from PSUM directly.
   4. The positive logit is computed exactly on the side: dma_gather of the
      1024 target label rows (f32), Square-accum norms -> stats -> g_tgt /
      yt_tgt, and an f32 dot with the staged text rows on Pool.
   5. After all groups: max8 over 32 candidates -> top-2 negatives; one
      tensor_scalar folds hx_m in; batched Exp/Ln tail -> per-sample loss.
"""

from contextlib import ExitStack

import numpy as np

import concourse.bass as bass
import concourse.tile as tile
from concourse import bacc, mybir
from concourse import bass_utils

F32 = mybir.dt.float32
BF16 = mybir.dt.bfloat16
I32 = mybir.dt.int32
I16 = mybir.dt.int16
U16 = mybir.dt.uint16
AF = mybir.ActivationFunctionType
ALU = mybir.AluOpType
AX = mybir.AxisListType

N_CORES = 8
M_FULL = 8192
C = 4096
D = 768
M_LOC = M_FULL // N_CORES   # 1024 rows per core
P = 128
NT_M = M_LOC // P           # 8 m-tiles
NT_C = C // P               # 32 label tiles
KCH = D // P                # 6 contraction chunks
NG = 4                      # label groups
GT = NT_C // NG             # 8 label tiles per group
GC = C // NG                # 1024 labels per group
NF = 512
EPS = 1e-8
MASK = -448.0               # onehot mask weight (exact in bf16)


def _stats_chain(nc, pool, nsq_raw, w, alpha_b, curv_b, isqch_b, tg,
                 want_t=False, want_tq=False):
    """From ||raw_row||^2 (128,w) compute gfac = alpha*sinh(rc)/rc plus the
    time component t = cosh(rc)/sqrt(curv) and/or tq = t/gfac, where
    rc = sqrt(curv)*alpha*||raw_row||.  ACT funcs are Exp/Ln only (single
    table); 1/x goes through the exact DVE reciprocal."""
    a2 = pool.tile([P, 1], F32, tag=f"a2{tg}")
    nc.vector.tensor_mul(a2[:], alpha_b[:], alpha_b[:])
    nsq = pool.tile([P, w], F32, tag=f"nsq{tg}")
    nc.vector.tensor_scalar(nsq[:], nsq_raw[:], a2[:], None, op0=ALU.mult)
    rc2 = pool.tile([P, w], F32, tag=f"rc2{tg}")
    nc.vector.tensor_scalar(rc2[:], nsq[:], curv_b[:], None, op0=ALU.mult)
    nc.vector.tensor_scalar_max(rc2[:], rc2[:], EPS * EPS)
    lr2 = pool.tile([P, w], F32, tag=f"lr2{tg}")
    nc.scalar.activation(lr2[:], rc2[:], AF.Ln)
    rc = pool.tile([P, w], F32, tag=f"rc{tg}")
    nc.scalar.activation(rc[:], lr2[:], AF.Exp, scale=0.5)
    rinv = pool.tile([P, w], F32, tag=f"rinv{tg}")
    nc.scalar.activation(rinv[:], lr2[:], AF.Exp, scale=-0.5)
    ep = pool.tile([P, w], F32, tag=f"ep{tg}")
    nc.scalar.activation(ep[:], rc[:], AF.Exp)
    en = pool.tile([P, w], F32, tag=f"en{tg}")
    nc.scalar.activation(en[:], rc[:], AF.Exp, scale=-1.0)
    sh = pool.tile([P, w], F32, tag=f"sh{tg}")
    nc.vector.tensor_sub(sh[:], ep[:], en[:])
    nc.vector.tensor_scalar_mul(sh[:], sh[:], 0.5)
    fac = pool.tile([P, w], F32, tag=f"fac{tg}")
    nc.vector.tensor_mul(fac[:], sh[:], rinv[:])
    gfac = pool.tile([P, w], F32, tag=f"gfac{tg}")
    nc.vector.tensor_scalar(gfac[:], fac[:], alpha_b[:], None, op0=ALU.mult)
    t = None
    if want_t or want_tq:
        # x_time = cosh(dist-from-origin)/sqrt(curv) on the hyperboloid
        t = pool.tile([P, w], F32, tag=f"t{tg}")
        nc.vector.tensor_add(t[:], ep[:], en[:])
        nc.vector.tensor_scalar(t[:], t[:], isqch_b[:], None, op0=ALU.mult)
    tq = None
    if want_tq:
        ginv = pool.tile([P, w], F32, tag=f"ginv{tg}")
        nc.vector.reciprocal(ginv[:], gfac[:])
        tq = pool.tile([P, w], F32, tag=f"tq{tg}")
        nc.vector.tensor_mul(tq[:], t[:], ginv[:])
    return gfac, t, tq


def _split_hi_lo(nc, pool, x, w, tg):
    """Split f32 (128,w) into exact bf16 hi + lo halves."""
    hi = pool.tile([P, w], BF16, tag=f"hi{tg}")
    nc.vector.tensor_copy(hi[:], x[:])
    hif = pool.tile([P, w], F32, tag=f"hif{tg}")
    nc.vector.tensor_copy(hif[:], hi[:])
    lof = pool.tile([P, w], F32, tag=f"lof{tg}")
    nc.vector.tensor_sub(lof[:], x[:], hif[:])
    lo = pool.tile([P, w], BF16, tag=f"lo{tg}")
    nc.vector.tensor_copy(lo[:], lof[:])
    return hi, lo


def build_kernel(ctx: ExitStack, tc: tile.TileContext, nt_m: int = NT_M):
    nc = tc.nc

    text_d = nc.dram_tensor("text_embeddings", (M_LOC, D), F32, kind="ExternalInput").ap()
    labels_d = nc.dram_tensor("label_embeddings", (C, D), F32, kind="ExternalInput").ap()
    tgt_d = nc.dram_tensor("target_labels", (M_LOC, 1), I32, kind="ExternalInput").ap()
    curv_log_d = nc.dram_tensor("curv_log", (1, 1), F32, kind="ExternalInput").ap()
    ta_log_d = nc.dram_tensor("text_alpha_log", (1, 1), F32, kind="ExternalInput").ap()
    la_log_d = nc.dram_tensor("label_alpha_log", (1, 1), F32, kind="ExternalInput").ap()
    loss_d = nc.dram_tensor("loss", (M_LOC, 1), F32, kind="ExternalOutput").ap()

    const = ctx.enter_context(tc.tile_pool(name="const", bufs=1))
    tiny = ctx.enter_context(tc.tile_pool(name="tiny", bufs=2))
    junk = ctx.enter_context(tc.tile_pool(name="junk", bufs=2))
    lstage = ctx.enter_context(tc.tile_pool(name="lstage", bufs=2))
    ypool = ctx.enter_context(tc.tile_pool(name="ypool", bufs=2))
    psum = ctx.enter_context(tc.tile_pool(name="psum", bufs=4, space="PSUM"))
    dram = ctx.enter_context(tc.tile_pool(name="dram", bufs=1, space="DRAM"))

    # ---- runtime scalars on the ACT ring (SP ring starts with text) ----
    def bload(ap_d, tag):
        b = const.tile([P, 1], F32, tag=tag)
        nc.scalar.dma_start(b[:], bass.AP(ap_d.tensor, 0, [[0, P], [1, 1]]))
        return b

    cl_b = bload(curv_log_d, "cl_b")
    ta_b = bload(ta_log_d, "ta_b")
    la_b = bload(la_log_d, "la_b")
    curv_b = const.tile([P, 1], F32, tag="curv_b")
    nc.scalar.activation(curv_b[:], cl_b[:], AF.Exp)
    at_b = const.tile([P, 1], F32, tag="at_b")
    nc.scalar.activation(at_b[:], ta_b[:], AF.Exp)
    al_b = const.tile([P, 1], F32, tag="al_b")
    nc.scalar.activation(al_b[:], la_b[:], AF.Exp)
    isqc_b = const.tile([P, 1], F32, tag="isqc_b")
    nc.scalar.activation(isqc_b[:], cl_b[:], AF.Exp, scale=-0.5)
    isqch_b = const.tile([P, 1], F32, tag="isqch_b")  # 0.5/sqrt(curv)
    nc.vector.tensor_scalar_mul(isqch_b[:], isqc_b[:], 0.5)
    ncurv_b = const.tile([P, 1], F32, tag="ncurv_b")
    nc.vector.tensor_scalar_mul(ncurv_b[:], curv_b[:], -1.0)

    # ---- constants + targets (small; issued after text/g0 on their rings)
    iota_u = const.tile([P, C], U16, tag="iota_u")
    nc.gpsimd.iota(iota_u[:], [[1, C]], channel_multiplier=0)
    iota_col = const.tile([P, 1], F32, tag="iota_col")
    nc.gpsimd.iota(
        iota_col[:], [[0, 1]], channel_multiplier=1,
        allow_small_or_imprecise_dtypes=True,
    )
    # negI[p, i] = -448 * (i == p): stationary weights of the mask matmul
    negI = const.tile([P, P], BF16, tag="negI")
    nc.vector.tensor_scalar(
        negI[:], iota_u[:, 0:P], iota_col[:], MASK, op0=ALU.is_equal,
        op1=ALU.mult,
    )
    # posI: exact 0/1 identity, the permutation operand of PE transposes
    posI = const.tile([P, P], BF16, tag="posI")
    nc.vector.tensor_scalar(
        posI[:], iota_u[:, 0:P], iota_col[:], None, op0=ALU.is_equal,
    )

    # ---- text pass first: its transpose chain gates the first matmul ----
    # input loads go on the SP ring; scratch writes + transposes + reads go
    # on the ACT ring so big input loads never head-of-line block them.
    xstage = const.tile([P, NT_M, D], F32, tag="xstage")
    xbf = lstage.tile([P, GT, D], BF16, tag="ybf")  # shares the ybf slot
    def load_group(g, hp=False):
        c0 = g * GC
        src = labels_d[c0:c0 + GC, :].rearrange("(a p) d -> p a d", p=P)
        lab = lstage.tile([P, GT, D], F32, tag="lab")
        with tc.high_priority() if hp else ExitStack():
            for q in range(4):
                qs = slice(q * GT // 4, (q + 1) * GT // 4)
                nc.sync.dma_start(lab[:, qs, :], src[:, qs, :])
        return lab

    nh = (nt_m + 1) // 2
    sls = [slice(0, nh), slice(nh, nt_m)]
    for sl in sls:
        if sl.stop > sl.start:
            nc.sync.dma_start(
                xstage[:, sl, :],
                text_d[sl.start * P:sl.stop * P, :].rearrange(
                    "(a p) d -> p a d", p=P))
    # first label group load queues on SP right behind the text halves;
    # the xscr/xT chain below overlaps its drain
    lab_tiles = [load_group(0, hp=True)]
    for sl in sls:
        if sl.stop > sl.start:
            with tc.high_priority():
                nc.scalar.activation(xbf[:, sl, :], xstage[:, sl, :], AF.Copy)
    # K-major text tiles via PE transposes (PE is idle in the preamble;
    # avoids the bf16 DRAM round-trip + XBAR transposes on the serial DMA)
    xT_all = []
    with tc.high_priority():
        for k in range(KCH):
            pst = psum.tile([P, M_LOC], BF16, tag="pst", bufs=2)
            for a in range(nt_m):
                nc.tensor.transpose(
                    pst[:, a * P:(a + 1) * P],
                    xbf[:, a, k * P:(k + 1) * P], posI[:])
            t = const.tile([P, M_LOC], BF16, tag=f"xTall{k}")
            nc.vector.tensor_copy(t[:, 0:nt_m * P], pst[:, 0:nt_m * P])
            xT_all.append(t)

    # text norms + stats (ACT stream continues after the xbf copies)
    nsqx_raw = const.tile([P, NT_M], F32, tag="nsqx_raw")
    jx = junk.tile([P, D], F32, tag="jD")
    for a in range(nt_m):
        nc.scalar.activation(
            jx[:], xstage[:, a, :], AF.Square,
            accum_out=nsqx_raw[:, a:a + 1],
        )
    hx, _, xtq = _stats_chain(
        nc, const, nsqx_raw, NT_M, at_b, curv_b, isqch_b, "x", want_tq=True)
    xth, xtl = _split_hi_lo(nc, const, xtq, NT_M, "x")
    xcol_ap = [[1, P], [P, NT_M]]
    xthscr = dram.tile([M_LOC, 1], BF16, tag="xthscr")
    xtlscr = dram.tile([M_LOC, 1], BF16, tag="xtlscr")
    xtime3 = const.tile([3, M_LOC], BF16, tag="xtime3")
    with tc.high_priority():
        nc.sync.dma_start(bass.AP(xthscr[:].tensor, 0, xcol_ap), xth[:])
        nc.sync.dma_start(bass.AP(xtlscr[:].tensor, 0, xcol_ap), xtl[:])
        nc.sync.dma_start(xtime3[0:1, :], xthscr[:].rearrange("a b -> b a"))
        nc.sync.dma_start(xtime3[1:2, :], xthscr[:].rearrange("a b -> b a"))
        nc.sync.dma_start(xtime3[2:3, :], xtlscr[:].rearrange("a b -> b a"))

    eps24 = const.tile([P, 3 * NT_M], F32, tag="eps24")
    nc.gpsimd.memset(eps24[:], 1.0 + EPS)



    tgt_all = const.tile([P, NT_M], I32, tag="tgt_all")
    nc.sync.dma_start(tgt_all[:], bass.AP(tgt_d.tensor, 0, [[1, P], [P, NT_M]]))
    tgt_f = const.tile([P, NT_M], F32, tag="tgt_f")
    nc.vector.tensor_copy(tgt_f[:], tgt_all[:])

    # persistent per-m-tile state across label groups
    cand = const.tile([P, NT_M * NG * 8], F32, tag="cand")
    nsqy_raw = const.tile([P, NT_C], F32, tag="nsqy_raw")
    yscr = dram.tile([C, D], BF16, tag="yscr")
    ythscr = dram.tile([C, 1], BF16, tag="ythscr")
    ytlscr = dram.tile([C, 1], BF16, tag="ytlscr")

    # ---- label groups: load -> norms -> stats -> g-folded bf16 -> yT ----
    for g in range(NG):
        c0 = g * GC
        gsl = slice(g * GT, (g + 1) * GT)
        lab = lab_tiles[g]
        # f32->bf16 on Pool (plain copy) while ACT runs the norms
        ybf = lstage.tile([P, GT, D], BF16, tag="ybf")
        nc.gpsimd.tensor_copy(ybf[:, 0:GT // 2, :], lab[:, 0:GT // 2, :])
        nc.gpsimd.tensor_copy(ybf[:, GT // 2:, :], lab[:, GT // 2:, :])
        jy = junk.tile([P, D], F32, tag="jD")
        for a in range(GT):
            nc.scalar.activation(
                jy[:], lab[:, a, :], AF.Square,
                accum_out=nsqy_raw[:, g * GT + a:g * GT + a + 1],
            )
        g_col, yt_col, _ = _stats_chain(
            nc, tiny, nsqy_raw[:, gsl], GT, al_b, curv_b, isqch_b, f"y{g}",
            want_t=True)
        ytn = tiny.tile([P, GT], F32, tag=f"ytn{g}")
        nc.vector.tensor_scalar_mul(ytn[:], yt_col[:], -1.0)
        yth_g, ytl_g = _split_hi_lo(nc, tiny, ytn, GT, f"y{g}")

        # fold g_c in-place on DVE at 4x rate (all-bf16 SBUF operands), then
        # stream this group's scratch + transposes at high priority so later
        # prefetch loads can't starve the chain that gates the matmuls
        yTg = ypool.tile([P, KCH, GC], BF16, tag="yTg")
        ytrow = ypool.tile([3, GC], BF16, tag="ytrow")
        with tc.high_priority():
            for a in range(GT):
                nc.vector.tensor_scalar(
                    ybf[:, a, :], ybf[:, a, :], g_col[:, a:a + 1], None,
                    op0=ALU.mult,
                )
            if g <= 1:
                # first groups via PE transposes: no serial-DMA round-trip,
                # so the first matmuls start ~15us earlier
                for k in range(KCH):
                    pst = psum.tile([P, M_LOC], BF16, tag="pst", bufs=2)
                    for a in range(GT):
                        nc.tensor.transpose(
                            pst[:, a * P:(a + 1) * P],
                            ybf[:, a, k * P:(k + 1) * P], posI[:])
                    nc.vector.tensor_copy(yTg[:, k, :], pst[:, 0:GC])
            else:
                nc.sync.dma_start(
                    yscr[c0:c0 + GC, :].rearrange("(a p) d -> p a d", p=P),
                    ybf[:],
                )
                # K-major label tiles via XBAR DMA transpose
                for k in range(KCH):
                    nc.sync.dma_start_transpose(
                        yTg[:, k, :], yscr[c0:c0 + GC, k * P:(k + 1) * P])

            # time rows for this group via tiny DRAM round-trip
            col_ap = [[1, P], [P, GT]]
            nc.sync.dma_start(bass.AP(ythscr[:].tensor, c0, col_ap), yth_g[:])
            nc.sync.dma_start(bass.AP(ytlscr[:].tensor, c0, col_ap), ytl_g[:])
            nc.sync.dma_start(
                ytrow[0:1, :], ythscr[c0:c0 + GC, :].rearrange("a b -> b a"))
            nc.sync.dma_start(
                ytrow[1:2, :], ytlscr[c0:c0 + GC, :].rearrange("a b -> b a"))
            nc.sync.dma_start(
                ytrow[2:3, :], ythscr[c0:c0 + GC, :].rearrange("a b -> b a"))

        # prefetch up to two groups ahead so each group's post-load chain
        # (norms -> stats -> fold -> yscr -> transposes) has a full PE
        # quantum to complete before its matmuls are due
        while len(lab_tiles) < min(g + 2, NG):
            lab_tiles.append(load_group(len(lab_tiles)))

        if g == 2:
            # positive-pair gather: issue now so it drains during group 3's
            # matmuls and vpos is ready before the merge.  idxs: the (16, 64)
            # target block replicated into every 16-partition group (each of
            # the 8 GPSIMD Q7 cores reads only its own SBUF slice).
            tgt_w = const.tile([P, M_LOC // 16], I32, tag="tgt_w")
            for k in range(8):
                nc.sync.dma_start(
                    tgt_w[16 * k:16 * (k + 1), :],
                    bass.AP(tgt_d.tensor, 0, [[1, 16], [16, M_LOC // 16]]))
            idxs = const.tile([P, M_LOC // 16], I16, tag="idxs")
            nc.vector.tensor_copy(idxs[:], tgt_w[:])
            # dma_gather's Q7 ucode lives in the `mlp` library
            from concourse import library_config
            nc.gpsimd.load_library(library_config.mlp)
            ygat = const.tile([P, NT_M, D], F32, tag="ygat")
            nc.gpsimd.dma_gather(
                ygat[:], labels_d[:, :], idxs[:], M_LOC, M_LOC, D,
                elem_step=D,
            )
            nc.gpsimd.load_library(library_config.standard)

        # ---- main loop for this group ----
        for m in range(nt_m):
            rows = slice(m * P, (m + 1) * P)
            # onehot row block for the positive mask (DVE 4x: all 2-byte)
            oh = tiny.tile([P, GC], BF16, tag="oh")
            nc.vector.tensor_scalar(
                oh[:], iota_u[:, c0:c0 + GC], tgt_f[:, m:m + 1], None,
                op0=ALU.is_equal,
            )
            ps = psum.tile([P, GC], F32, tag="ps", bufs=3)
            for h in range(2):
                hs = slice(h * NF, (h + 1) * NF)
                for k in range(KCH):
                    nc.tensor.matmul(
                        ps[:, hs], xT_all[k][:, rows], yTg[:, k, hs],
                        start=(k == 0), stop=False,
                    )
                nc.tensor.matmul(
                    ps[:, hs], xtime3[:, rows], ytrow[:, hs],
                    start=False, stop=False,
                )
                nc.tensor.matmul(
                    ps[:, hs], negI[:], oh[:, hs],
                    start=False, stop=True,
                )
            # top-8 candidates of this group's W block (PSUM read)
            ci = (m * NG + g) * 8
            nc.vector.max(cand[:, ci:ci + 8], ps[:])

    # (positive-pair gather was issued inside the g==2 iteration)

    nsqp_raw = const.tile([P, NT_M], F32, tag="nsqp_raw")
    jp = junk.tile([P, D], F32, tag="jD2")
    for a in range(nt_m):
        nc.scalar.activation(
            jp[:], ygat[:, a, :], AF.Square,
            accum_out=nsqp_raw[:, a:a + 1],
        )
    g_tgt, yt_tgt, _ = _stats_chain(
        nc, const, nsqp_raw, NT_M, al_b, curv_b, isqch_b, "p", want_t=True)
    s_pos = const.tile([P, NT_M], F32, tag="s_pos")
    for a in range(nt_m):
        jd = junk.tile([P, D], F32, tag="jP")
        nc.vector.scalar_tensor_tensor(
            jd[:], xstage[:, a, :], 1.0, ygat[:, a, :],
            op0=ALU.mult, op1=ALU.mult,
            accum_out=s_pos[:, a:a + 1],
        )
    # vpos = g_tgt * s_pos - xtq * yt_tgt   (inner_pos / hx)
    vpos = const.tile([P, NT_M], F32, tag="vpos")
    nc.vector.tensor_mul(vpos[:], g_tgt[:], s_pos[:])
    xtyt = const.tile([P, NT_M], F32, tag="xtyt")
    nc.vector.tensor_mul(xtyt[:], xtq[:], yt_tgt[:])
    nc.vector.tensor_sub(vpos[:], vpos[:], xtyt[:])

    # ---- per-m-tile: merge groups, top-2 negatives, fold hx ----
    V_all = const.tile([P, 3 * NT_M], F32, tag="V_all")
    for m in range(nt_m):
        top8 = tiny.tile([P, 8], F32, tag="top8")
        nc.vector.max(top8[:], cand[:, m * NG * 8:(m + 1) * NG * 8])
        v3 = tiny.tile([P, 3], F32, tag="v3")
        nc.vector.tensor_copy(v3[:, 0:1], vpos[:, m:m + 1])
        nc.vector.tensor_copy(v3[:, 1:3], top8[:, 0:2])
        # fold hx_m (row-constant, positive) back in: inner = hx * W
        nc.vector.tensor_scalar(
            V_all[:, 3 * m:3 * m + 3], v3[:], hx[:, m:m + 1], None,
            op0=ALU.mult,
        )

    # ---- batched loss tail over all m-tiles (Exp/Ln only) ----
    W = 3 * nt_m
    cd = const.tile([P, 3 * NT_M], F32, tag="cd")
    nc.vector.scalar_tensor_tensor(
        cd[:, :W], V_all[:, :W], ncurv_b[:], eps24[:, :W],
        op0=ALU.mult, op1=ALU.max,
    )
    sq = const.tile([P, 3 * NT_M], F32, tag="sqv")
    nc.vector.tensor_mul(sq[:, :W], cd[:, :W], cd[:, :W])
    nc.vector.tensor_scalar_add(sq[:, :W], sq[:, :W], -1.0)
    lsq = const.tile([P, 3 * NT_M], F32, tag="lsq")
    nc.scalar.activation(lsq[:, :W], sq[:, :W], AF.Ln)
    root = const.tile([P, 3 * NT_M], F32, tag="root")
    nc.scalar.activation(root[:, :W], lsq[:, :W], AF.Exp, scale=0.5)
    u = const.tile([P, 3 * NT_M], F32, tag="u")
    nc.vector.tensor_add(u[:, :W], cd[:, :W], root[:, :W])
    dist = const.tile([P, 3 * NT_M], F32, tag="dist")
    nc.scalar.activation(dist[:, :W], u[:, :W], AF.Ln)
    nc.vector.tensor_scalar(dist[:, :W], dist[:, :W], isqc_b[:], None, op0=ALU.mult)
    ev = const.tile([P, 3 * NT_M], F32, tag="ev")
    nc.scalar.activation(ev[:, :W], dist[:, :W], AF.Exp, scale=-1.0)
    s8 = const.tile([P, NT_M], F32, tag="s8")
    ev3 = ev[:, :W].rearrange("p (m k) -> p m k", k=3)
    nc.vector.tensor_reduce(s8[:, :nt_m], ev3, axis=AX.X, op=ALU.add)
    lg = const.tile([P, NT_M], F32, tag="lg")
    nc.scalar.activation(lg[:, :nt_m], s8[:, :nt_m], AF.Ln)
    loss_all = const.tile([P, NT_M], F32, tag="loss_all")
    dist_pos = dist[:, :W].rearrange("p (m k) -> p m k", k=3)[:, :, 0:1]
    nc.vector.tensor_add(
        loss_all[:, :nt_m], lg[:, :nt_m],
        dist_pos.rearrange("p m k -> p (m k)"),
    )
    nc.sync.dma_start(
        bass.AP(loss_d.tensor, 0, [[1, P], [P, nt_m]]), loss_all[:, :nt_m]
    )


_CACHED = {}


def _compile_single_act_table(nc):
    """Compile with the act-table insertion pass steered to the one table
    that serves every ACT func this kernel uses (exp/ln/square/copy all live
    in `natural_log_exp_and_others`).  The pass picks the first table
    containing each func, which otherwise thrashes between the exp-only and
    ln-only tables (1.3us per reload).  Table ids stay global, so the NEFF
    loads the real combined table — hardware-correct."""
    import concourse.bacc as bacc_mod
    orig = bacc_mod.get_activation_tables
    keep = "natural_log_exp_and_others"
    ours = {AF.Exp, AF.Ln, AF.Square, AF.Copy}

    def patched(arch):
        tabs = orig(arch)
        return {
            name: (set(s) if name == keep else set(s) - ours)
            for name, s in tabs.items()
        }

    bacc_mod.get_activation_tables = patched
    try:
        nc.compile()
    finally:
        bacc_mod.get_activation_tables = orig


def build_program(nt_m: int = NT_M):
    if nt_m not in _CACHED:
        nc = bacc.Bacc(
            "TRN2",
            target_bir_lowering=False,
            debug=False,
            enable_asserts=False,
            num_devices=N_CORES,
        )
        with tile.TileContext(nc) as tc, ExitStack() as ctx:
            build_kernel(ctx, tc, nt_m)
        _compile_single_act_table(nc)
        _CACHED[nt_m] = nc
    return _CACHED[nt_m]


def shard_inputs(inputs) -> list[dict[str, np.ndarray]]:
    text = np.ascontiguousarray(np.asarray(inputs["text_embeddings"], np.float32))
    labels = np.ascontiguousarray(np.asarray(inputs["label_embeddings"], np.float32))
    tgt = np.asarray(inputs["target_labels"]).astype(np.int32).reshape(M_FULL, 1)
    s11 = lambda v: np.asarray(v, np.float32).reshape(1, 1)
    curv_log = s11(inputs["curv_log"])
    ta = s11(inputs["text_alpha_log"])
    la = s11(inputs["label_alpha_log"])
    in_maps = []
    for i in range(N_CORES):
        r = slice(i * M_LOC, (i + 1) * M_LOC)
        in_maps.append({
            "text_embeddings": np.ascontiguousarray(text[r]),
            "label_embeddings": labels,
            "target_labels": np.ascontiguousarray(tgt[r]),
            "curv_log": curv_log,
            "text_alpha_log": ta,
            "label_alpha_log": la,
        })
    return in_maps


def run_sharded(inputs, trace=False, nt_m: int = NT_M, **kwargs):
    nc = build_program(nt_m)
    in_maps = shard_inputs(inputs)
    res = bass_utils.run_bass_kernel_spmd(
        nc, in_maps, core_ids=list(range(N_CORES)), trace=trace, **kwargs
    )
    loss = np.concatenate(
        [res.results[i]["loss"].reshape(M_LOC) for i in range(N_CORES)]
    ).astype(np.float32)
    return loss, res


def kernel(**inputs) -> np.ndarray:
    loss, _ = run_sharded(inputs, trace=False)
    return loss



# revision 14
# speedup vs baseline: 1.3407x; 1.3407x over previous
"""Trainium2 Bass kernel for the MERU-style hyperbolic contrastive loss.

Problem (hardcoded shapes):
  text_embeddings (8192, 768) f32, label_embeddings (4096, 768) f32,
  target_labels (8192,) int32, three scalar log-params.
  Output: loss (8192,) f32 per-sample.

Sharding: data-parallel over text rows across 8 NeuronCores (1024 rows each);
label_embeddings and scalars replicated.

Per-core algorithm (v3 — fp8 DoubleRow matmuls + positive-hoist mask):
  Lorentz inner product factors as  inner[m,c] = hx_m * W[m,c] with
      W[m,c] = g_c*S_raw[m,c] - xtq_m*yt_c,
  where S_raw = raw_text @ raw_labels^T, hx_m/g_c the exp-map scale factors,
  yt_c the label time component and xtq_m = x_time_m / hx_m.  Since hx_m > 0
  is row-constant, per-row top-k runs directly on (SY*W) in PSUM.

   1. Text: one staged f32 load; per-row norms; Sqrt-free stats (Exp/Ln only,
      single ACT table); f32->fp8e4m3 convert on ACT; K-major fp8 tiles via
      PE transposes (PE idle in the preamble).
   2. Labels in 4 pipelined groups of 1024: grouped DMA load, norms on ACT,
      group stats, then a single Pool tensor_scalar fuses the g_c*SY scale
      with the f32->fp8 conversion; K-major fp8 tiles via PE transposes.
   3. Main loop per (group, m-tile): PSUM accumulates SY*W via 2x(3 fp8
      DoubleRow K=256 matmuls + bf16 rank-3 time term + bf16 mask matmul).
      The mask adds +448*SY*onehot(target), hoisting the positive to the
      top-1 slot: one DVE max8 per (group, m-tile) then yields both the
      top negatives (slots 1+) and the positive (slot0 - 448*SY) — no
      label gather / separate positive path needed.  SY=16 scales the fp8
      label quantization out of the subnormal range.
   4. After all groups: max8 over 32 candidates -> [pos+mask, neg1, neg2];
      subtract the mask, fold hx_m/SY in; batched Exp/Ln tail -> loss.
"""

from contextlib import ExitStack

import numpy as np

import concourse.bass as bass
import concourse.tile as tile
from concourse import bacc, mybir
from concourse import bass_utils

F32 = mybir.dt.float32
BF16 = mybir.dt.bfloat16
FP8 = mybir.dt.float8e4
I32 = mybir.dt.int32
U16 = mybir.dt.uint16
AF = mybir.ActivationFunctionType
ALU = mybir.AluOpType
AX = mybir.AxisListType
DR = mybir.MatmulPerfMode.DoubleRow

N_CORES = 8
M_FULL = 8192
C = 4096
D = 768
M_LOC = M_FULL // N_CORES   # 1024 rows per core
P = 128
NT_M = M_LOC // P           # 8 m-tiles
NT_C = C // P               # 32 label tiles
KCH = D // P                # 6 contraction chunks
NG = 4                      # label groups
GT = NT_C // NG             # 8 label tiles per group
GC = C // NG                # 1024 labels per group
NF = 512
EPS = 1e-8
SY = 16.0                   # fp8 label scale (dodges e4m3 subnormals)
MASK = 448.0 * SY           # positive-hoist mask weight (exact in bf16)


def _stats_chain(nc, pool, nsq_raw, w, alpha_b, curv_b, isqch_b, tg,
                 want_t=False, want_tq=False):
    """From ||raw_row||^2 (128,w) compute gfac = alpha*sinh(rc)/rc plus the
    time component t = cosh(rc)/sqrt(curv) and/or tq = t/gfac, where
    rc = sqrt(curv)*alpha*||raw_row||.  ACT funcs are Exp/Ln only (single
    table); 1/x goes through the exact DVE reciprocal."""
    a2 = pool.tile([P, 1], F32, tag=f"a2{tg}")
    nc.vector.tensor_mul(a2[:], alpha_b[:], alpha_b[:])
    nsq = pool.tile([P, w], F32, tag=f"nsq{tg}")
    nc.vector.tensor_scalar(nsq[:], nsq_raw[:], a2[:], None, op0=ALU.mult)
    rc2 = pool.tile([P, w], F32, tag=f"rc2{tg}")
    nc.vector.tensor_scalar(rc2[:], nsq[:], curv_b[:], None, op0=ALU.mult)
    nc.vector.tensor_scalar_max(rc2[:], rc2[:], EPS * EPS)
    lr2 = pool.tile([P, w], F32, tag=f"lr2{tg}")
    nc.scalar.activation(lr2[:], rc2[:], AF.Ln)
    rc = pool.tile([P, w], F32, tag=f"rc{tg}")
    nc.scalar.activation(rc[:], lr2[:], AF.Exp, scale=0.5)
    rinv = pool.tile([P, w], F32, tag=f"rinv{tg}")
    nc.scalar.activation(rinv[:], lr2[:], AF.Exp, scale=-0.5)
    ep = pool.tile([P, w], F32, tag=f"ep{tg}")
    nc.scalar.activation(ep[:], rc[:], AF.Exp)
    en = pool.tile([P, w], F32, tag=f"en{tg}")
    nc.scalar.activation(en[:], rc[:], AF.Exp, scale=-1.0)
    sh = pool.tile([P, w], F32, tag=f"sh{tg}")
    nc.vector.tensor_sub(sh[:], ep[:], en[:])
    nc.vector.tensor_scalar_mul(sh[:], sh[:], 0.5)
    fac = pool.tile([P, w], F32, tag=f"fac{tg}")
    nc.vector.tensor_mul(fac[:], sh[:], rinv[:])
    gfac = pool.tile([P, w], F32, tag=f"gfac{tg}")
    nc.vector.tensor_scalar(gfac[:], fac[:], alpha_b[:], None, op0=ALU.mult)
    t = None
    if want_t or want_tq:
        # x_time = cosh(dist-from-origin)/sqrt(curv) on the hyperboloid
        t = pool.tile([P, w], F32, tag=f"t{tg}")
        nc.vector.tensor_add(t[:], ep[:], en[:])
        nc.vector.tensor_scalar(t[:], t[:], isqch_b[:], None, op0=ALU.mult)
    tq = None
    if want_tq:
        ginv = pool.tile([P, w], F32, tag=f"ginv{tg}")
        nc.vector.reciprocal(ginv[:], gfac[:])
        tq = pool.tile([P, w], F32, tag=f"tq{tg}")
        nc.vector.tensor_mul(tq[:], t[:], ginv[:])
    return gfac, t, tq


def _split3_fp8(nc, pool, x, w, tg):
    """Split f32 (128,w) into three fp8e4m3 levels h+m+l (residual coding)."""
    levels = []
    res = x
    for lv in range(3):
        q = pool.tile([P, w], FP8, tag=f"q{lv}{tg}")
        nc.vector.tensor_copy(q[:], res[:])
        levels.append(q)
        if lv < 2:
            qf = pool.tile([P, w], F32, tag=f"qf{lv}{tg}")
            nc.vector.tensor_copy(qf[:], q[:])
            nres = pool.tile([P, w], F32, tag=f"qr{lv}{tg}")
            nc.vector.tensor_sub(nres[:], res[:], qf[:])
            res = nres
    return levels


def build_kernel(ctx: ExitStack, tc: tile.TileContext, nt_m: int = NT_M):
    nc = tc.nc

    text_d = nc.dram_tensor("text_embeddings", (M_LOC, D), F32, kind="ExternalInput").ap()
    labels_d = nc.dram_tensor("label_embeddings", (C, D), F32, kind="ExternalInput").ap()
    tgt_d = nc.dram_tensor("target_labels", (M_LOC, 1), I32, kind="ExternalInput").ap()
    curv_log_d = nc.dram_tensor("curv_log", (1, 1), F32, kind="ExternalInput").ap()
    ta_log_d = nc.dram_tensor("text_alpha_log", (1, 1), F32, kind="ExternalInput").ap()
    la_log_d = nc.dram_tensor("label_alpha_log", (1, 1), F32, kind="ExternalInput").ap()
    loss_d = nc.dram_tensor("loss", (M_LOC, 1), F32, kind="ExternalOutput").ap()

    const = ctx.enter_context(tc.tile_pool(name="const", bufs=1))
    tiny = ctx.enter_context(tc.tile_pool(name="tiny", bufs=2))
    junk = ctx.enter_context(tc.tile_pool(name="junk", bufs=2))
    lstage = ctx.enter_context(tc.tile_pool(name="lstage", bufs=2))
    ypool = ctx.enter_context(tc.tile_pool(name="ypool", bufs=2))
    psum = ctx.enter_context(tc.tile_pool(name="psum", bufs=4, space="PSUM"))
    dram = ctx.enter_context(tc.tile_pool(name="dram", bufs=1, space="DRAM"))

    # ---- runtime scalars on the ACT ring (SP ring starts with text) ----
    def bload(ap_d, tag):
        b = const.tile([P, 1], F32, tag=tag)
        nc.scalar.dma_start(b[:], bass.AP(ap_d.tensor, 0, [[0, P], [1, 1]]))
        return b

    cl_b = bload(curv_log_d, "cl_b")
    ta_b = bload(ta_log_d, "ta_b")
    la_b = bload(la_log_d, "la_b")
    curv_b = const.tile([P, 1], F32, tag="curv_b")
    nc.scalar.activation(curv_b[:], cl_b[:], AF.Exp)
    at_b = const.tile([P, 1], F32, tag="at_b")
    nc.scalar.activation(at_b[:], ta_b[:], AF.Exp)
    al_b = const.tile([P, 1], F32, tag="al_b")
    nc.scalar.activation(al_b[:], la_b[:], AF.Exp)
    isqc_b = const.tile([P, 1], F32, tag="isqc_b")
    nc.scalar.activation(isqc_b[:], cl_b[:], AF.Exp, scale=-0.5)
    isqch_b = const.tile([P, 1], F32, tag="isqch_b")  # 0.5/sqrt(curv)
    nc.vector.tensor_scalar_mul(isqch_b[:], isqc_b[:], 0.5)
    ncurv_b = const.tile([P, 1], F32, tag="ncurv_b")
    nc.vector.tensor_scalar_mul(ncurv_b[:], curv_b[:], -1.0)

    # ---- constants + targets (small; issued after text/g0 on their rings)
    iota_u = const.tile([P, C], U16, tag="iota_u")
    nc.gpsimd.iota(iota_u[:], [[1, C]], channel_multiplier=0)
    iota_col = const.tile([P, 1], F32, tag="iota_col")
    nc.gpsimd.iota(
        iota_col[:], [[0, 1]], channel_multiplier=1,
        allow_small_or_imprecise_dtypes=True,
    )
    # maskI[p, i] = +MASK * (i == p): stationary weights of the mask matmul
    # (hoists the positive to the global top-1 candidate slot)
    maskI = const.tile([P, P], BF16, tag="maskI")
    nc.vector.tensor_scalar(
        maskI[:], iota_u[:, 0:P], iota_col[:], MASK, op0=ALU.is_equal,
        op1=ALU.mult,
    )
    # posI: exact 0/1 identity, the permutation operand of PE transposes
    posI = const.tile([P, P], BF16, tag="posI")
    nc.vector.tensor_scalar(
        posI[:], iota_u[:, 0:P], iota_col[:], None, op0=ALU.is_equal,
    )

    # ---- text pass first: its transpose chain gates the first matmul ----
    xstage = const.tile([P, NT_M, D], F32, tag="xstage")
    xbf8 = const.tile([P, NT_M, D], BF16, tag="xbf8")

    def load_group(g, hp=False):
        c0 = g * GC
        src = labels_d[c0:c0 + GC, :].rearrange("(a p) d -> p a d", p=P)
        lab = lstage.tile([P, GT, D], F32, tag="lab")
        with tc.high_priority() if hp else ExitStack():
            for q in range(4):
                qs = slice(q * GT // 4, (q + 1) * GT // 4)
                nc.sync.dma_start(lab[:, qs, :], src[:, qs, :])
        return lab

    nh = (nt_m + 1) // 2
    sls = [slice(0, nh), slice(nh, nt_m)]
    for sl in sls:
        if sl.stop > sl.start:
            nc.sync.dma_start(
                xstage[:, sl, :],
                text_d[sl.start * P:sl.stop * P, :].rearrange(
                    "(a p) d -> p a d", p=P))
    # first label group load queues on SP right behind the text halves
    lab_tiles = [load_group(0, hp=True)]
    for sl in sls:
        if sl.stop > sl.start:
            with tc.high_priority():
                nc.gpsimd.tensor_copy(xbf8[:, sl, :], xstage[:, sl, :])
    # K-major fp8 text tiles via PE transposes (PE idle in the preamble)
    xT_all = const.tile([P, KCH, M_LOC], FP8, tag="xT_all")
    with tc.high_priority():
        for k in range(KCH):
            pst = psum.tile([P, M_LOC], BF16, tag="pst", bufs=2)
            for a in range(nt_m):
                nc.tensor.transpose(
                    pst[:, a * P:(a + 1) * P],
                    xbf8[:, a, k * P:(k + 1) * P], posI[:])
            if k % 2 == 0:
                nc.scalar.activation(
                    xT_all[:, k, 0:nt_m * P], pst[:, 0:nt_m * P], AF.Copy)
            else:
                nc.vector.tensor_copy(xT_all[:, k, 0:nt_m * P], pst[:, 0:nt_m * P])

    # text norms striped across DVE/Pool (ACT busy with converts) + stats
    nsqx_raw = const.tile([P, NT_M], F32, tag="nsqx_raw")
    jx = junk.tile([P, D], F32, tag="jD")
    for a in range(nt_m):
        if a % 2 == 0:
            nc.scalar.activation(
                jx[:], xstage[:, a, :], AF.Square,
                accum_out=nsqx_raw[:, a:a + 1],
            )
        else:
            nc.vector.scalar_tensor_tensor(
                jx[:], xstage[:, a, :], 1.0, xstage[:, a, :],
                op0=ALU.mult, op1=ALU.mult,
                accum_out=nsqx_raw[:, a:a + 1],
            )
    hx, _, xtq = _stats_chain(
        nc, const, nsqx_raw, NT_M, at_b, curv_b, isqch_b, "x", want_tq=True)
    # hx_s folds the 1/SY candidate rescale into the row-constant factor
    hx_s = const.tile([P, NT_M], F32, tag="hx_s")
    nc.vector.tensor_scalar_mul(hx_s[:], hx[:], 1.0 / SY)
    # xtq -> 3 fp8 levels; rank-6 rows [i=0: (h,m,l); i=1: (h,m,h)]
    xlv = _split3_fp8(nc, const, xtq, NT_M, "x")
    xcol_ap = [[1, P], [P, NT_M]]
    xscr = [dram.tile([M_LOC, 1], FP8, tag=f"xt6scr{v}", name=f"xt6scr{v}")
            for v in range(3)]
    xtime6 = const.tile([3, 2, M_LOC], FP8, tag="xtime6")
    with tc.high_priority():
        for v in range(3):
            nc.sync.dma_start(bass.AP(xscr[v][:].tensor, 0, xcol_ap), xlv[v][:])
        for r, v in ((0, 0), (1, 1), (2, 2)):
            nc.sync.dma_start(
                xtime6[r:r + 1, 0, :], xscr[v][:].rearrange("a b -> b a"))
        for r, v in ((0, 0), (1, 1), (2, 0)):
            nc.sync.dma_start(
                xtime6[r:r + 1, 1, :], xscr[v][:].rearrange("a b -> b a"))

    eps24 = const.tile([P, 3 * NT_M], F32, tag="eps24")
    nc.gpsimd.memset(eps24[:], 1.0 + EPS)

    tgt_all = const.tile([P, NT_M], I32, tag="tgt_all")
    nc.sync.dma_start(tgt_all[:], bass.AP(tgt_d.tensor, 0, [[1, P], [P, NT_M]]))
    tgt_f = const.tile([P, NT_M], F32, tag="tgt_f")
    nc.vector.tensor_copy(tgt_f[:], tgt_all[:])

    # persistent per-m-tile state across label groups
    cand = const.tile([P, NT_M * NG * 8], F32, tag="cand")
    nsqy_raw = const.tile([P, NT_C], F32, tag="nsqy_raw")
    yscr = [dram.tile([C, 1], FP8, tag=f"yt6scr{v}", name=f"yt6scr{v}")
            for v in range(3)]

    # ---- label groups: load -> norms -> stats -> fused fold+fp8 -> yT ----
    for g in range(NG):
        c0 = g * GC
        gsl = slice(g * GT, (g + 1) * GT)
        lab = lab_tiles[g]
        jy = junk.tile([P, D], F32, tag="jD")
        for a in range(GT):
            if a % 2 == 0:
                nc.scalar.activation(
                    jy[:], lab[:, a, :], AF.Square,
                    accum_out=nsqy_raw[:, g * GT + a:g * GT + a + 1],
                )
            else:
                nc.vector.scalar_tensor_tensor(
                    jy[:], lab[:, a, :], 1.0, lab[:, a, :],
                    op0=ALU.mult, op1=ALU.mult,
                    accum_out=nsqy_raw[:, g * GT + a:g * GT + a + 1],
                )
        g_col, yt_col, _ = _stats_chain(
            nc, tiny, nsqy_raw[:, gsl], GT, al_b, curv_b, isqch_b, f"y{g}",
            want_t=True)
        gs_col = tiny.tile([P, GT], F32, tag=f"gs{g}")
        nc.vector.tensor_scalar_mul(gs_col[:], g_col[:], SY)
        ytn = tiny.tile([P, GT], F32, tag=f"ytn{g}")
        nc.vector.tensor_scalar_mul(ytn[:], yt_col[:], -SY)
        ylv = _split3_fp8(nc, tiny, ytn, GT, f"y{g}")

        # fused (SY*g_c)-scale + f32->fp8 conversion on ACT, then the
        # group's K-major tiles via PE transposes at high priority so later
        # prefetch loads can't starve the chain that gates the matmuls
        ybf8 = lstage.tile([P, GT, D], BF16, tag="ybf8")
        yTg = ypool.tile([P, KCH, GC], FP8, tag="yTg")
        ytrow6 = ypool.tile([3, 2, GC], FP8, tag="ytrow6")
        with tc.high_priority():
            # time rows first: the DRAM round-trip drains while the folds
            # and transposes run
            col_ap = [[1, P], [P, GT]]
            for v in range(3):
                nc.sync.dma_start(
                    bass.AP(yscr[v][:].tensor, c0, col_ap), ylv[v][:])
            for r, v in ((0, 0), (1, 0), (2, 0)):
                nc.sync.dma_start(
                    ytrow6[r:r + 1, 0, :],
                    yscr[v][c0:c0 + GC, :].rearrange("a b -> b a"))
            for r, v in ((0, 1), (1, 1), (2, 2)):
                nc.sync.dma_start(
                    ytrow6[r:r + 1, 1, :],
                    yscr[v][c0:c0 + GC, :].rearrange("a b -> b a"))

            for a in range(GT):
                nc.scalar.activation(
                    ybf8[:, a, :], lab[:, a, :], AF.Copy,
                    scale=gs_col[:, a:a + 1],
                )
            for k in range(KCH):
                pst = psum.tile([P, GC], BF16, tag="pst", bufs=2)
                for a in range(GT):
                    nc.tensor.transpose(
                        pst[:, a * P:(a + 1) * P],
                        ybf8[:, a, k * P:(k + 1) * P], posI[:])
                if k % 2 == 0:
                    nc.scalar.activation(yTg[:, k, :], pst[:, 0:GC], AF.Copy)
                else:
                    nc.vector.tensor_copy(yTg[:, k, :], pst[:, 0:GC])

        # prefetch up to two groups ahead so each group's post-load chain
        # has a full PE quantum to complete before its matmuls are due
        while len(lab_tiles) < min(g + 2, NG):
            lab_tiles.append(load_group(len(lab_tiles)))

        # onehot row blocks for the positive-hoist mask, all built before
        # the m-loop so PE never waits on DVE mid-loop (4x: all 2-byte)
        oh_all = tiny.tile([P, NT_M, GC], BF16, tag="oh_all")
        for m in range(nt_m):
            nc.vector.tensor_scalar(
                oh_all[:, m, :], iota_u[:, c0:c0 + GC], tgt_f[:, m:m + 1],
                None, op0=ALU.is_equal,
            )

        # ---- main loop for this group ----
        for m in range(nt_m):
            rows = slice(m * P, (m + 1) * P)
            ps = psum.tile([P, GC], F32, tag="ps", bufs=3)
            for h in range(2):
                hs = slice(h * NF, (h + 1) * NF)
                for j in range(KCH // 2):
                    nc.tensor.matmul(
                        ps[:, hs], xT_all[:, 2 * j:2 * j + 2, rows],
                        yTg[:, 2 * j:2 * j + 2, hs],
                        start=(j == 0), stop=False, perf_mode=DR,
                    )
                nc.tensor.matmul(
                    ps[:, hs], xtime6[:, :, rows], ytrow6[:, :, hs],
                    start=False, stop=False, perf_mode=DR,
                )
                nc.tensor.matmul(
                    ps[:, hs], maskI[:], oh_all[:, m, hs],
                    start=False, stop=True,
                )
            # top-8 candidates of this group's SY*W block (PSUM read)
            ci = (m * NG + g) * 8
            nc.vector.max(cand[:, ci:ci + 8], ps[:])

    # ---- per-m-tile: merge groups -> [pos+MASK, neg1, neg2], fold hx ----
    V_all = const.tile([P, 3 * NT_M], F32, tag="V_all")
    for m in range(nt_m):
        top8 = tiny.tile([P, 8], F32, tag="top8")
        nc.vector.max(top8[:], cand[:, m * NG * 8:(m + 1) * NG * 8])
        v3 = tiny.tile([P, 3], F32, tag="v3")
        nc.vector.tensor_scalar_add(v3[:, 0:1], top8[:, 0:1], -MASK)
        nc.vector.tensor_copy(v3[:, 1:3], top8[:, 1:3])
        # fold hx_m/SY (row-constant, positive) back in: inner = hx * W
        nc.vector.tensor_scalar(
            V_all[:, 3 * m:3 * m + 3], v3[:], hx_s[:, m:m + 1], None,
            op0=ALU.mult,
        )

    # ---- batched loss tail over all m-tiles (Exp/Ln only) ----
    W = 3 * nt_m
    cd = const.tile([P, 3 * NT_M], F32, tag="cd")
    nc.vector.scalar_tensor_tensor(
        cd[:, :W], V_all[:, :W], ncurv_b[:], eps24[:, :W],
        op0=ALU.mult, op1=ALU.max,
    )
    sq = const.tile([P, 3 * NT_M], F32, tag="sqv")
    nc.vector.tensor_mul(sq[:, :W], cd[:, :W], cd[:, :W])
    nc.vector.tensor_scalar_add(sq[:, :W], sq[:, :W], -1.0)
    lsq = const.tile([P, 3 * NT_M], F32, tag="lsq")
    nc.scalar.activation(lsq[:, :W], sq[:, :W], AF.Ln)
    root = const.tile([P, 3 * NT_M], F32, tag="root")
    nc.scalar.activation(root[:, :W], lsq[:, :W], AF.Exp, scale=0.5)
    u = const.tile([P, 3 * NT_M], F32, tag="u")
    nc.vector.tensor_add(u[:, :W], cd[:, :W], root[:, :W])
    dist = const.tile([P, 3 * NT_M], F32, tag="dist")
    nc.scalar.activation(dist[:, :W], u[:, :W], AF.Ln)
    nc.vector.tensor_scalar(dist[:, :W], dist[:, :W], isqc_b[:], None, op0=ALU.mult)
    ev = const.tile([P, 3 * NT_M], F32, tag="ev")
    nc.scalar.activation(ev[:, :W], dist[:, :W], AF.Exp, scale=-1.0)
    s8 = const.tile([P, NT_M], F32, tag="s8")
    ev3 = ev[:, :W].rearrange("p (m k) -> p m k", k=3)
    nc.vector.tensor_reduce(s8[:, :nt_m], ev3, axis=AX.X, op=ALU.add)
    lg = const.tile([P, NT_M], F32, tag="lg")
    nc.scalar.activation(lg[:, :nt_m], s8[:, :nt_m], AF.Ln)
    loss_all = const.tile([P, NT_M], F32, tag="loss_all")
    dist_pos = dist[:, :W].rearrange("p (m k) -> p m k", k=3)[:, :, 0:1]
    nc.vector.tensor_add(
        loss_all[:, :nt_m], lg[:, :nt_m],
        dist_pos.rearrange("p m k -> p (m k)"),
    )
    nc.sync.dma_start(
        bass.AP(loss_d.tensor, 0, [[1, P], [P, nt_m]]), loss_all[:, :nt_m]
    )


_CACHED = {}


def _compile_single_act_table(nc):
    """Compile with the act-table insertion pass steered to the one table
    that serves every ACT func this kernel uses (exp/ln/square/copy all live
    in `natural_log_exp_and_others`).  The pass picks the first table
    containing each func, which otherwise thrashes between the exp-only and
    ln-only tables (1.3us per reload).  Table ids stay global, so the NEFF
    loads the real combined table — hardware-correct."""
    import concourse.bacc as bacc_mod
    orig = bacc_mod.get_activation_tables
    keep = "natural_log_exp_and_others"
    ours = {AF.Exp, AF.Ln, AF.Square, AF.Copy}

    def patched(arch):
        tabs = orig(arch)
        return {
            name: (set(s) if name == keep else set(s) - ours)
            for name, s in tabs.items()
        }

    bacc_mod.get_activation_tables = patched
    try:
        nc.compile()
    finally:
        bacc_mod.get_activation_tables = orig


def build_program(nt_m: int = NT_M):
    if nt_m not in _CACHED:
        nc = bacc.Bacc(
            "TRN2",
            target_bir_lowering=False,
            debug=False,
            enable_asserts=False,
            num_devices=N_CORES,
        )
        with tile.TileContext(nc) as tc, ExitStack() as ctx:
            build_kernel(ctx, tc, nt_m)
        _compile_single_act_table(nc)
        _CACHED[nt_m] = nc
    return _CACHED[nt_m]


def shard_inputs(inputs) -> list[dict[str, np.ndarray]]:
    text = np.ascontiguousarray(np.asarray(inputs["text_embeddings"], np.float32))
    labels = np.ascontiguousarray(np.asarray(inputs["label_embeddings"], np.float32))
    tgt = np.asarray(inputs["target_labels"]).astype(np.int32).reshape(M_FULL, 1)
    s11 = lambda v: np.asarray(v, np.float32).reshape(1, 1)
    curv_log = s11(inputs["curv_log"])
    ta = s11(inputs["text_alpha_log"])
    la = s11(inputs["label_alpha_log"])
    in_maps = []
    for i in range(N_CORES):
        r = slice(i * M_LOC, (i + 1) * M_LOC)
        in_maps.append({
            "text_embeddings": np.ascontiguousarray(text[r]),
            "label_embeddings": labels,
            "target_labels": np.ascontiguousarray(tgt[r]),
            "curv_log": curv_log,
            "text_alpha_log": ta,
            "label_alpha_log": la,
        })
    return in_maps


def run_sharded(inputs, trace=False, nt_m: int = NT_M, **kwargs):
    nc = build_program(nt_m)
    in_maps = shard_inputs(inputs)
    res = bass_utils.run_bass_kernel_spmd(
        nc, in_maps, core_ids=list(range(N_CORES)), trace=trace, **kwargs
    )
    loss = np.concatenate(
        [res.results[i]["loss"].reshape(M_LOC) for i in range(N_CORES)]
    ).astype(np.float32)
    return loss, res


def kernel(**inputs) -> np.ndarray:
    loss, _ = run_sharded(inputs, trace=False)
    return loss


# revision 30
# speedup vs baseline: 1.5007x; 1.1194x over previous
"""Trainium2 Bass kernel for the MERU-style hyperbolic contrastive loss.

Problem (hardcoded shapes):
  text_embeddings (8192, 768) f32, label_embeddings (4096, 768) f32,
  target_labels (8192,) int32, three scalar log-params.
  Output: loss (8192,) f32 per-sample.

Sharding: data-parallel over text rows across 8 NeuronCores (1024 rows each);
label_embeddings and scalars replicated.

Per-core algorithm (v3 — fp8 DoubleRow matmuls + positive-hoist mask):
  Lorentz inner product factors as  inner[m,c] = hx_m * W[m,c] with
      W[m,c] = g_c*S_raw[m,c] - xtq_m*yt_c,
  where S_raw = raw_text @ raw_labels^T, hx_m/g_c the exp-map scale factors,
  yt_c the label time component and xtq_m = x_time_m / hx_m.  Since hx_m > 0
  is row-constant, per-row top-k runs directly on (SY*W) in PSUM.

   1. Text: one staged f32 load; per-row norms; Sqrt-free stats (Exp/Ln only,
      single ACT table); f32->fp8e4m3 convert on ACT; K-major fp8 tiles via
      PE transposes (PE idle in the preamble).
   2. Labels in 4 pipelined groups of 1024: grouped DMA load, norms on ACT,
      group stats, then a single Pool tensor_scalar fuses the g_c*SY scale
      with the f32->fp8 conversion; K-major fp8 tiles via PE transposes.
   3. Main loop per (group, m-tile): PSUM accumulates SY*W via 2x(3 fp8
      DoubleRow K=256 matmuls + bf16 rank-3 time term + bf16 mask matmul).
      The mask adds +448*SY*onehot(target), hoisting the positive to the
      top-1 slot: one DVE max8 per (group, m-tile) then yields both the
      top negatives (slots 1+) and the positive (slot0 - 448*SY) — no
      label gather / separate positive path needed.  SY=16 scales the fp8
      label quantization out of the subnormal range.
   4. After all groups: max8 over 32 candidates -> [pos+mask, neg1, neg2];
      subtract the mask, fold hx_m/SY in; batched Exp/Ln tail -> loss.
"""

from contextlib import ExitStack

import numpy as np

import concourse.bass as bass
import concourse.tile as tile
from concourse import bacc, mybir
from concourse import bass_utils

F32 = mybir.dt.float32
BF16 = mybir.dt.bfloat16
FP8 = mybir.dt.float8e4
I32 = mybir.dt.int32
U16 = mybir.dt.uint16
AF = mybir.ActivationFunctionType
ALU = mybir.AluOpType
AX = mybir.AxisListType
DR = mybir.MatmulPerfMode.DoubleRow

N_CORES = 8
M_FULL = 8192
C = 4096
D = 768
M_LOC = M_FULL // N_CORES   # 1024 rows per core
P = 128
NT_M = M_LOC // P           # 8 m-tiles
NT_C = C // P               # 32 label tiles
KCH = D // P                # 6 contraction chunks
NG = 4                      # label groups
GT = NT_C // NG             # 8 label tiles per group
GC = C // NG                # 1024 labels per group
NF = 512
EPS = 1e-8
SY = 16.0                   # fp8 label scale (dodges e4m3 subnormals)
MASK = 448.0 * SY           # positive-hoist mask weight (exact in bf16)


def _stats_chain(nc, pool, nsq_raw, w, alpha_b, curv_b, isqch_b, tg,
                 want_t=False, want_tq=False):
    """From ||raw_row||^2 (128,w) compute gfac = alpha*sinh(rc)/rc plus the
    time component t = cosh(rc)/sqrt(curv) and/or tq = t/gfac, where
    rc = sqrt(curv)*alpha*||raw_row||.  ACT funcs are Exp/Ln only (single
    table); 1/x goes through the exact DVE reciprocal."""
    a2 = pool.tile([P, 1], F32, tag=f"a2{tg}")
    nc.vector.tensor_mul(a2[:], alpha_b[:], alpha_b[:])
    nsq = pool.tile([P, w], F32, tag=f"nsq{tg}")
    nc.vector.tensor_scalar(nsq[:], nsq_raw[:], a2[:], None, op0=ALU.mult)
    rc2 = pool.tile([P, w], F32, tag=f"rc2{tg}")
    nc.vector.tensor_scalar(rc2[:], nsq[:], curv_b[:], None, op0=ALU.mult)
    nc.vector.tensor_scalar_max(rc2[:], rc2[:], EPS * EPS)
    lr2 = pool.tile([P, w], F32, tag=f"lr2{tg}")
    nc.scalar.activation(lr2[:], rc2[:], AF.Ln)
    rc = pool.tile([P, w], F32, tag=f"rc{tg}")
    nc.scalar.activation(rc[:], lr2[:], AF.Exp, scale=0.5)
    rinv = pool.tile([P, w], F32, tag=f"rinv{tg}")
    nc.scalar.activation(rinv[:], lr2[:], AF.Exp, scale=-0.5)
    ep = pool.tile([P, w], F32, tag=f"ep{tg}")
    nc.scalar.activation(ep[:], rc[:], AF.Exp)
    en = pool.tile([P, w], F32, tag=f"en{tg}")
    nc.scalar.activation(en[:], rc[:], AF.Exp, scale=-1.0)
    sh = pool.tile([P, w], F32, tag=f"sh{tg}")
    nc.vector.tensor_sub(sh[:], ep[:], en[:])
    nc.vector.tensor_scalar_mul(sh[:], sh[:], 0.5)
    fac = pool.tile([P, w], F32, tag=f"fac{tg}")
    nc.vector.tensor_mul(fac[:], sh[:], rinv[:])
    gfac = pool.tile([P, w], F32, tag=f"gfac{tg}")
    nc.vector.tensor_scalar(gfac[:], fac[:], alpha_b[:], None, op0=ALU.mult)
    t = None
    if want_t or want_tq:
        # x_time = cosh(dist-from-origin)/sqrt(curv) on the hyperboloid
        t = pool.tile([P, w], F32, tag=f"t{tg}")
        nc.vector.tensor_add(t[:], ep[:], en[:])
        nc.vector.tensor_scalar(t[:], t[:], isqch_b[:], None, op0=ALU.mult)
    tq = None
    if want_tq:
        ginv = pool.tile([P, w], F32, tag=f"ginv{tg}")
        nc.vector.reciprocal(ginv[:], gfac[:])
        tq = pool.tile([P, w], F32, tag=f"tq{tg}")
        nc.vector.tensor_mul(tq[:], t[:], ginv[:])
    return gfac, t, tq


def _split3_fp8(nc, pool, x, w, tg):
    """Split f32 (128,w) into three fp8e4m3 levels h+m+l (residual coding)."""
    levels = []
    res = x
    for lv in range(3):
        q = pool.tile([P, w], FP8, tag=f"q{lv}{tg}")
        nc.vector.tensor_copy(q[:], res[:])
        levels.append(q)
        if lv < 2:
            qf = pool.tile([P, w], F32, tag=f"qf{lv}{tg}")
            nc.vector.tensor_copy(qf[:], q[:])
            nres = pool.tile([P, w], F32, tag=f"qr{lv}{tg}")
            nc.vector.tensor_sub(nres[:], res[:], qf[:])
            res = nres
    return levels


def build_kernel(ctx: ExitStack, tc: tile.TileContext, nt_m: int = NT_M):
    nc = tc.nc

    text_d = nc.dram_tensor("text_embeddings", (M_LOC, D), F32, kind="ExternalInput").ap()
    labels_d = nc.dram_tensor("label_embeddings", (C, D), F32, kind="ExternalInput").ap()
    tgt_d = nc.dram_tensor("target_labels", (M_LOC, 1), I32, kind="ExternalInput").ap()
    curv_log_d = nc.dram_tensor("curv_log", (1, 1), F32, kind="ExternalInput").ap()
    ta_log_d = nc.dram_tensor("text_alpha_log", (1, 1), F32, kind="ExternalInput").ap()
    la_log_d = nc.dram_tensor("label_alpha_log", (1, 1), F32, kind="ExternalInput").ap()
    loss_d = nc.dram_tensor("loss", (M_LOC, 1), F32, kind="ExternalOutput").ap()

    const = ctx.enter_context(tc.tile_pool(name="const", bufs=1))
    tiny = ctx.enter_context(tc.tile_pool(name="tiny", bufs=2))
    junk = ctx.enter_context(tc.tile_pool(name="junk", bufs=2))
    lstage = ctx.enter_context(tc.tile_pool(name="lstage", bufs=2))
    ypool = ctx.enter_context(tc.tile_pool(name="ypool", bufs=2))
    psum = ctx.enter_context(tc.tile_pool(name="psum", bufs=4, space="PSUM"))
    dram = ctx.enter_context(tc.tile_pool(name="dram", bufs=1, space="DRAM"))

    # ---- runtime scalars on the ACT ring (SP ring starts with text) ----
    def bload(ap_d, tag):
        b = const.tile([P, 1], F32, tag=tag)
        nc.scalar.dma_start(b[:], bass.AP(ap_d.tensor, 0, [[0, P], [1, 1]]))
        return b

    cl_b = bload(curv_log_d, "cl_b")
    ta_b = bload(ta_log_d, "ta_b")
    la_b = bload(la_log_d, "la_b")
    curv_b = const.tile([P, 1], F32, tag="curv_b")
    nc.scalar.activation(curv_b[:], cl_b[:], AF.Exp)
    at_b = const.tile([P, 1], F32, tag="at_b")
    nc.scalar.activation(at_b[:], ta_b[:], AF.Exp)
    al_b = const.tile([P, 1], F32, tag="al_b")
    nc.scalar.activation(al_b[:], la_b[:], AF.Exp)
    isqc_b = const.tile([P, 1], F32, tag="isqc_b")
    nc.scalar.activation(isqc_b[:], cl_b[:], AF.Exp, scale=-0.5)
    isqch_b = const.tile([P, 1], F32, tag="isqch_b")  # 0.5/sqrt(curv)
    nc.vector.tensor_scalar_mul(isqch_b[:], isqc_b[:], 0.5)
    ncurv_b = const.tile([P, 1], F32, tag="ncurv_b")
    nc.vector.tensor_scalar_mul(ncurv_b[:], curv_b[:], -1.0)

    # ---- constants + targets (small; issued after text/g0 on their rings)
    iota_u = const.tile([P, C], U16, tag="iota_u")
    nc.gpsimd.iota(iota_u[:], [[1, C]], channel_multiplier=0)
    iota_col = const.tile([P, 1], F32, tag="iota_col")
    nc.gpsimd.iota(
        iota_col[:], [[0, 1]], channel_multiplier=1,
        allow_small_or_imprecise_dtypes=True,
    )
    # maskI[p, i] = +MASK * (i == p): stationary weights of the mask matmul
    # (hoists the positive to the global top-1 candidate slot)
    maskI = const.tile([P, P], BF16, tag="maskI")
    nc.vector.tensor_scalar(
        maskI[:], iota_u[:, 0:P], iota_col[:], MASK, op0=ALU.is_equal,
        op1=ALU.mult,
    )
    # posI: exact 0/1 identity, the permutation operand of PE transposes
    posI = const.tile([P, P], BF16, tag="posI")
    nc.vector.tensor_scalar(
        posI[:], iota_u[:, 0:P], iota_col[:], None, op0=ALU.is_equal,
    )

    # ---- text pass first: its transpose chain gates the first matmul ----
    xstage = const.tile([P, NT_M, D], F32, tag="xstage")
    xbf8 = const.tile([P, NT_M, D], BF16, tag="xbf8")

    def load_group(g, hp=False):
        c0 = g * GC
        src = labels_d[c0:c0 + GC, :].rearrange("(a p) d -> p a d", p=P)
        quarters = []
        with tc.high_priority() if hp else ExitStack():
            for q in range(4):
                labq = lstage.tile([P, GT // 4, D], F32, tag="lab", bufs=8)
                qs = slice(q * GT // 4, (q + 1) * GT // 4)
                nc.sync.dma_start(labq[:], src[:, qs, :])
                quarters.append(labq)
        return quarters

    nq = max(1, nt_m // 4)
    sls = [slice(i, min(i + nq, nt_m)) for i in range(0, nt_m, nq)]
    for sl in sls:
        nc.sync.dma_start(
            xstage[:, sl, :],
            text_d[sl.start * P:sl.stop * P, :].rearrange(
                "(a p) d -> p a d", p=P))
    # first label group load queues on SP right behind the text quarters
    lab_tiles = [load_group(0)]
    for sl in sls:
        with tc.high_priority():
            nc.gpsimd.tensor_copy(xbf8[:, sl, :], xstage[:, sl, :])
    # K-major fp8 text tiles via PE identity-matmul transposes (regular
    # matmul mode: bf16 in, f32 PSUM out, fp8 made by the copies)
    xT_all = const.tile([P, KCH, M_LOC], FP8, tag="xT_all")
    nhh = (nt_m + 3) // 4
    with tc.high_priority():
        for k in range(KCH):
            for h in range(nhh):
                na = min(4, nt_m - 4 * h)
                pst = psum.tile([P, NF], F32, tag="pst", bufs=2)
                for i in range(na):
                    a = 4 * h + i
                    nc.tensor.matmul(
                        pst[:, i * P:(i + 1) * P],
                        xbf8[:, a, k * P:(k + 1) * P], posI[:],
                        start=True, stop=True)
                dst = xT_all[:, k, 4 * h * P:(4 * h + na) * P]
                nc.scalar.activation(dst, pst[:, 0:na * P], AF.Copy)

    # text norms on DVE from the bf16 copy (ACT is busy with xT copies)
    nsqx_raw = const.tile([P, NT_M], F32, tag="nsqx_raw")
    jx = junk.tile([P, D], BF16, tag="jDb")
    for a in range(nt_m):
        nc.vector.scalar_tensor_tensor(
            jx[:], xbf8[:, a, :], 1.0, xbf8[:, a, :],
            op0=ALU.mult, op1=ALU.mult,
            accum_out=nsqx_raw[:, a:a + 1],
        )
    hx, _, xtq = _stats_chain(
        nc, const, nsqx_raw, NT_M, at_b, curv_b, isqch_b, "x", want_tq=True)
    # hx_s folds the 1/SY candidate rescale into the row-constant factor
    hx_s = const.tile([P, NT_M], F32, tag="hx_s")
    nc.vector.tensor_scalar_mul(hx_s[:], hx[:], 1.0 / SY)
    # xtq -> 3 fp8 levels; rank-6 rows [i=0: (h,m,l); i=1: (h,m,h)]
    xlv = _split3_fp8(nc, const, xtq, NT_M, "x")
    xcol_ap = [[1, P], [P, NT_M]]
    xscr = [dram.tile([M_LOC, 1], FP8, tag=f"xt6scr{v}", name=f"xt6scr{v}")
            for v in range(3)]
    xtime6 = const.tile([3, 2, M_LOC], FP8, tag="xtime6")
    with tc.high_priority():
        for v in range(3):
            nc.sync.dma_start(bass.AP(xscr[v][:].tensor, 0, xcol_ap), xlv[v][:])
        for r, v in ((0, 0), (1, 1), (2, 2)):
            nc.sync.dma_start(
                xtime6[r:r + 1, 0, :], xscr[v][:].rearrange("a b -> b a"))
        for r, v in ((0, 0), (1, 1), (2, 0)):
            nc.sync.dma_start(
                xtime6[r:r + 1, 1, :], xscr[v][:].rearrange("a b -> b a"))

    eps24 = const.tile([P, 3 * NT_M], F32, tag="eps24")
    nc.gpsimd.memset(eps24[:], 1.0 + EPS)

    tgt_all = const.tile([P, NT_M], I32, tag="tgt_all")
    nc.sync.dma_start(tgt_all[:], bass.AP(tgt_d.tensor, 0, [[1, P], [P, NT_M]]))
    tgt_f = const.tile([P, NT_M], F32, tag="tgt_f")
    nc.vector.tensor_copy(tgt_f[:], tgt_all[:])

    # persistent per-m-tile state across label groups
    cand = const.tile([P, NT_M * NG * 8], F32, tag="cand")
    V_all = const.tile([P, 3 * NT_M], F32, tag="V_all")
    nsqy_raw = const.tile([P, NT_C], F32, tag="nsqy_raw")
    yscr = [dram.tile([C, 1], FP8, tag=f"yt6scr{v}", name=f"yt6scr{v}")
            for v in range(3)]

    # ---- label groups: load -> norms -> stats -> fused fold+fp8 -> yT ----
    for g in range(NG):
        c0 = g * GC
        gsl = slice(g * GT, (g + 1) * GT)
        labq = lab_tiles[g]
        nq4 = GT // 4
        ybf8 = lstage.tile([P, GT, D], BF16, tag="ybf8")
        for q in range(4):
            nc.gpsimd.tensor_copy(
                ybf8[:, q * nq4:(q + 1) * nq4, :], labq[q][:])
        jy = junk.tile([P, D], F32, tag="jD")
        jyb = junk.tile([P, D], BF16, tag="jDb")
        with tc.high_priority():
            for a in range(GT):
                if a % 8 >= 5:
                    nc.vector.scalar_tensor_tensor(
                        jyb[:], ybf8[:, a, :], 1.0, ybf8[:, a, :],
                        op0=ALU.mult, op1=ALU.mult,
                        accum_out=nsqy_raw[:, g * GT + a:g * GT + a + 1],
                    )
                else:
                    nc.scalar.activation(
                        jy[:], labq[a // nq4][:, a % nq4, :], AF.Square,
                        accum_out=nsqy_raw[:, g * GT + a:g * GT + a + 1],
                    )
        with tc.high_priority():
            g_col, yt_col, _ = _stats_chain(
                nc, tiny, nsqy_raw[:, gsl], GT, al_b, curv_b, isqch_b,
                f"y{g}", want_t=True)
            gs_col = tiny.tile([P, GT], F32, tag=f"gs{g}")
            nc.vector.tensor_scalar_mul(gs_col[:], g_col[:], SY)
            ytn = tiny.tile([P, GT], F32, tag=f"ytn{g}")
            nc.vector.tensor_scalar_mul(ytn[:], yt_col[:], -SY)
            ylv = _split3_fp8(nc, tiny, ytn, GT, f"y{g}")

        diagG = tiny.tile([P, GT, P], BF16, tag="diagG")
        for a in range(GT):
            nc.vector.tensor_scalar(
                diagG[:, a, :], posI[:], gs_col[:, a:a + 1], None,
                op0=ALU.mult,
            )
        yTg = ypool.tile([P, KCH, GC], FP8, tag="yTg")
        ytrow6 = ypool.tile([3, 2, GC], FP8, tag="ytrow6")
        with tc.high_priority():
            # time rows first: the DRAM round-trip drains while the folds
            # and transposes run
            col_ap = [[1, P], [P, GT]]
            for v in range(3):
                nc.sync.dma_start(
                    bass.AP(yscr[v][:].tensor, c0, col_ap), ylv[v][:])
            for r, v in ((0, 0), (1, 0), (2, 0)):
                nc.sync.dma_start(
                    ytrow6[r:r + 1, 0, :],
                    yscr[v][c0:c0 + GC, :].rearrange("a b -> b a"))
            for r, v in ((0, 1), (1, 1), (2, 2)):
                nc.sync.dma_start(
                    ytrow6[r:r + 1, 1, :],
                    yscr[v][c0:c0 + GC, :].rearrange("a b -> b a"))

            for k in range(KCH):
                for h in range(2):
                    pst = psum.tile([P, NF], F32, tag="pst", bufs=2)
                    for i in range(4):
                        a = 4 * h + i
                        nc.tensor.matmul(
                            pst[:, i * P:(i + 1) * P],
                            ybf8[:, a, k * P:(k + 1) * P], diagG[:, a, :],
                            start=True, stop=True)
                    dst = yTg[:, k, h * NF:(h + 1) * NF]
                    if g <= 1 and (2 * k + h) % 3 == 2:
                        nc.vector.tensor_copy(dst, pst[:])
                    else:
                        nc.scalar.activation(dst, pst[:], AF.Copy)

        # onehot row blocks for the positive-hoist mask, all built before
        # the m-loop so PE never waits on DVE mid-loop (4x: all 2-byte)
        oh_all = tiny.tile([P, NT_M, GC], BF16, tag="oh_all")
        for m in range(nt_m):
            nc.vector.tensor_scalar(
                oh_all[:, m, :], iota_u[:, c0:c0 + GC], tgt_f[:, m:m + 1],
                None, op0=ALU.is_equal,
            )

        # prefetch up to two groups ahead so each group's post-load chain
        # has a full PE quantum to complete before its matmuls are due
        while len(lab_tiles) < min(g + 2, NG):
            lab_tiles.append(load_group(len(lab_tiles)))

        # ---- main loop for this group ----
        for m in range(nt_m):
            rows = slice(m * P, (m + 1) * P)
            ps = psum.tile([P, GC], F32, tag="ps", bufs=3)
            for h in range(2):
                hs = slice(h * NF, (h + 1) * NF)
                for j in range(KCH // 2):
                    nc.tensor.matmul(
                        ps[:, hs], xT_all[:, 2 * j:2 * j + 2, rows],
                        yTg[:, 2 * j:2 * j + 2, hs],
                        start=(j == 0), stop=False, perf_mode=DR,
                    )
                nc.tensor.matmul(
                    ps[:, hs], xtime6[:, :, rows], ytrow6[:, :, hs],
                    start=False, stop=False, perf_mode=DR,
                )
                nc.tensor.matmul(
                    ps[:, hs], maskI[:], oh_all[:, m, hs],
                    start=False, stop=True,
                )
            # top-8 candidates of this group's SY*W block (PSUM read)
            ci = (m * NG + g) * 8
            nc.vector.max(cand[:, ci:ci + 8], ps[:])
            if g == NG - 1:
                # merge groups -> [pos+MASK, neg1, neg2], fold hx_m/SY in
                top8 = tiny.tile([P, 8], F32, tag="top8")
                nc.vector.max(top8[:], cand[:, m * NG * 8:(m + 1) * NG * 8])
                v3 = tiny.tile([P, 3], F32, tag="v3")
                nc.vector.tensor_scalar_add(v3[:, 0:1], top8[:, 0:1], -MASK)
                nc.vector.tensor_copy(v3[:, 1:3], top8[:, 1:3])
                nc.vector.tensor_scalar(
                    V_all[:, 3 * m:3 * m + 3], v3[:], hx_s[:, m:m + 1], None,
                    op0=ALU.mult,
                )

    # ---- batched loss tail over all m-tiles (Exp/Ln only) ----
    W = 3 * nt_m
    cd = const.tile([P, 3 * NT_M], F32, tag="cd")
    nc.vector.scalar_tensor_tensor(
        cd[:, :W], V_all[:, :W], ncurv_b[:], eps24[:, :W],
        op0=ALU.mult, op1=ALU.max,
    )
    sq = const.tile([P, 3 * NT_M], F32, tag="sqv")
    nc.vector.tensor_mul(sq[:, :W], cd[:, :W], cd[:, :W])
    nc.vector.tensor_scalar_add(sq[:, :W], sq[:, :W], -1.0)
    lsq = const.tile([P, 3 * NT_M], F32, tag="lsq")
    nc.scalar.activation(lsq[:, :W], sq[:, :W], AF.Ln)
    root = const.tile([P, 3 * NT_M], F32, tag="root")
    nc.scalar.activation(root[:, :W], lsq[:, :W], AF.Exp, scale=0.5)
    u = const.tile([P, 3 * NT_M], F32, tag="u")
    nc.vector.tensor_add(u[:, :W], cd[:, :W], root[:, :W])
    dist = const.tile([P, 3 * NT_M], F32, tag="dist")
    nc.scalar.activation(dist[:, :W], u[:, :W], AF.Ln)
    nc.vector.tensor_scalar(dist[:, :W], dist[:, :W], isqc_b[:], None, op0=ALU.mult)
    ev = const.tile([P, 3 * NT_M], F32, tag="ev")
    nc.scalar.activation(ev[:, :W], dist[:, :W], AF.Exp, scale=-1.0)
    s8 = const.tile([P, NT_M], F32, tag="s8")
    ev3 = ev[:, :W].rearrange("p (m k) -> p m k", k=3)
    nc.vector.tensor_reduce(s8[:, :nt_m], ev3, axis=AX.X, op=ALU.add)
    lg = const.tile([P, NT_M], F32, tag="lg")
    nc.scalar.activation(lg[:, :nt_m], s8[:, :nt_m], AF.Ln)
    loss_all = const.tile([P, NT_M], F32, tag="loss_all")
    dist_pos = dist[:, :W].rearrange("p (m k) -> p m k", k=3)[:, :, 0:1]
    nc.vector.tensor_add(
        loss_all[:, :nt_m], lg[:, :nt_m],
        dist_pos.rearrange("p m k -> p (m k)"),
    )
    nc.sync.dma_start(
        bass.AP(loss_d.tensor, 0, [[1, P], [P, nt_m]]), loss_all[:, :nt_m]
    )


_CACHED = {}


def _compile_single_act_table(nc):
    """Compile with the act-table insertion pass steered to the one table
    that serves every ACT func this kernel uses (exp/ln/square/copy all live
    in `natural_log_exp_and_others`).  The pass picks the first table
    containing each func, which otherwise thrashes between the exp-only and
    ln-only tables (1.3us per reload).  Table ids stay global, so the NEFF
    loads the real combined table — hardware-correct."""
    import concourse.bacc as bacc_mod
    orig = bacc_mod.get_activation_tables
    keep = "natural_log_exp_and_others"
    ours = {AF.Exp, AF.Ln, AF.Square, AF.Copy}

    def patched(arch):
        tabs = orig(arch)
        return {
            name: (set(s) if name == keep else set(s) - ours)
            for name, s in tabs.items()
        }

    bacc_mod.get_activation_tables = patched
    try:
        nc.compile()
    finally:
        bacc_mod.get_activation_tables = orig


def build_program(nt_m: int = NT_M):
    if nt_m not in _CACHED:
        nc = bacc.Bacc(
            "TRN2",
            target_bir_lowering=False,
            debug=False,
            enable_asserts=False,
            num_devices=N_CORES,
        )
        with tile.TileContext(nc) as tc, ExitStack() as ctx:
            build_kernel(ctx, tc, nt_m)
        _compile_single_act_table(nc)
        _CACHED[nt_m] = nc
    return _CACHED[nt_m]


def shard_inputs(inputs) -> list[dict[str, np.ndarray]]:
    text = np.ascontiguousarray(np.asarray(inputs["text_embeddings"], np.float32))
    labels = np.ascontiguousarray(np.asarray(inputs["label_embeddings"], np.float32))
    tgt = np.asarray(inputs["target_labels"]).astype(np.int32).reshape(M_FULL, 1)
    s11 = lambda v: np.asarray(v, np.float32).reshape(1, 1)
    curv_log = s11(inputs["curv_log"])
    ta = s11(inputs["text_alpha_log"])
    la = s11(inputs["label_alpha_log"])
    in_maps = []
    for i in range(N_CORES):
        r = slice(i * M_LOC, (i + 1) * M_LOC)
        in_maps.append({
            "text_embeddings": np.ascontiguousarray(text[r]),
            "label_embeddings": labels,
            "target_labels": np.ascontiguousarray(tgt[r]),
            "curv_log": curv_log,
            "text_alpha_log": ta,
            "label_alpha_log": la,
        })
    return in_maps


def run_sharded(inputs, trace=False, nt_m: int = NT_M, **kwargs):
    nc = build_program(nt_m)
    in_maps = shard_inputs(inputs)
    res = bass_utils.run_bass_kernel_spmd(
        nc, in_maps, core_ids=list(range(N_CORES)), trace=trace, **kwargs
    )
    loss = np.concatenate(
        [res.results[i]["loss"].reshape(M_LOC) for i in range(N_CORES)]
    ).astype(np.float32)
    return loss, res


def kernel(**inputs) -> np.ndarray:
    loss, _ = run_sharded(inputs, trace=False)
    return loss
